# revision 1
# baseline (speedup 1.0000x reference)
"""NeRF volumetric alpha-compositing kernel for Trainium2 (Bass/Tile).

Full inputs:  rgbo [131072, 128, 4] f32, depth [131072, 128] f32.
Full output:  [131072, 3] f32.

Sharding: data-parallel over rays, 8 cores x 16384 rays.

Per-core algorithm (ray-per-partition layout; S=128 samples on free dim):
  delta[s]  = depth[s+1] - depth[s]            (DVE, shifted views)
  m[s]      = opacity[s] * delta[s]            (DVE; m[127] = opacity*1e9)
  cs        = inclusive_cumsum(m)              (DVE tensor_tensor_scan)
  t[0]      = 1;  t[i] = exp(-cs[i-1])         (ScalarE Exp, scale=-1)
  w[i]      = t[i] - t[i+1]                    (DVE)   [= T_i * alpha_i]
  out[c]    = sum_s w[s] * sigmoid(rgb[s,c])   (DVE tensor_tensor_reduce)
The last-sample FAR_DELTA=1e9 is exact: t[128]=exp(-cs[127]) underflows to 0
whenever opacity[127] > ~1e-7, else matches the reference expression.
"""

from contextlib import ExitStack

import numpy as np

import concourse.bass as bass
import concourse.tile as tile
from concourse import bacc, mybir
from concourse.bass_utils import run_bass_kernel_spmd

N_RAYS = 131072
S = 128
N_CORES = 8
NC_RAYS = N_RAYS // N_CORES  # 16384 rays per core
BLOCK = 128                  # rays per partition-block
F32 = mybir.dt.float32


def build_nerf_bass(
    n_rays: int = NC_RAYS,
    t_blocks: int = 8,
    gpsimd_delta_m: bool = False,
    gpsimd_channels: int = 0,
    repeat: int = 1,
    dma_only: bool = False,
    loop_iters: int = 0,
    skip: tuple = (),
    bufs: int = 2,
    scr_bufs: int = 4,
    dma_split: bool = False,
) -> bass.Bass:
    """Build the per-core Bass program for n_rays rays.

    gpsimd_delta_m: compute delta and m on GPSIMD instead of VectorE.
    gpsimd_channels: how many of the 3 weighted-reduce channels run as
      fused scalar_tensor_tensor(+accum) on GPSIMD instead of
      tensor_tensor_reduce on VectorE.
    """
    T = t_blocks
    SUPER = BLOCK * T
    assert n_rays % SUPER == 0
    n_super = n_rays // SUPER
    U = S + 4  # padded per-block stride for the t-table (129 used)

    nc = bacc.Bacc("TRN2", target_bir_lowering=False, debug=False)
    rgbo_h = nc.declare_dram_parameter("rgbo", [n_rays, S, 4], F32, isOutput=False)
    depth_h = nc.declare_dram_parameter("depth", [n_rays, S], F32, isOutput=False)
    out_h = nc.declare_dram_parameter("out", [n_rays, 3], F32, isOutput=True)

    rgbo_ap = rgbo_h.ap()
    depth_ap = depth_h.ap()
    out_ap = out_h.ap()

    with ExitStack() as ctx:
        tc = ctx.enter_context(tile.TileContext(nc))
        p_rgbo = ctx.enter_context(tc.tile_pool(name="rgbo", bufs=bufs))
        p_depth = ctx.enter_context(tc.tile_pool(name="depth", bufs=bufs))
        p_g = ctx.enter_context(tc.tile_pool(name="g", bufs=bufs))
        p_mid = ctx.enter_context(tc.tile_pool(name="mid", bufs=bufs))
        p_scr = ctx.enter_context(tc.tile_pool(name="scr", bufs=scr_bufs))
        p_out = ctx.enter_context(tc.tile_pool(name="outp", bufs=bufs))

        def emit_superblock(r0):
            rgbo_t = p_rgbo.tile([BLOCK, 4 * S * T], F32, tag="rgbo")
            rgbo_dst = rgbo_t.rearrange("p (t f) -> p t f", t=T)
            rgbo_src = rgbo_ap[r0 : r0 + SUPER].rearrange(
                "(p t) s c -> p t (s c)", p=BLOCK
            )
            depth_t = p_depth.tile([BLOCK, S * T], F32, tag="depth")
            depth_dst = depth_t.rearrange("p (t s) -> p t s", t=T)
            depth_src = depth_ap[r0 : r0 + SUPER].rearrange(
                "(p t) s -> p t s", p=BLOCK
            )
            if dma_split:
                h = T // 2
                nc.sync.dma_start(out=rgbo_dst[:, 0:h], in_=rgbo_src[:, 0:h])
                nc.scalar.dma_start(out=rgbo_dst[:, h:T], in_=rgbo_src[:, h:T])
                nc.gpsimd.dma_start(out=depth_dst, in_=depth_src)
            else:
                nc.sync.dma_start(out=rgbo_dst, in_=rgbo_src)
                nc.sync.dma_start(out=depth_dst, in_=depth_src)
            rgbo4 = rgbo_t.rearrange("p (t s c) -> p t s c", t=T, s=S, c=4)
            depth3 = depth_t.rearrange("p (t s) -> p t s", t=T)

            if dma_only:
                # consume both loads (prevents DCE), write output, skip compute
                out_t = p_out.tile([BLOCK, 3 * T], F32, tag="out")
                nc.vector.scalar_tensor_tensor(
                    out=out_t[:, 0:1], in0=rgbo_t[:, 0:1], scalar=0.0,
                    in1=depth_t[:, 0:1], op0=mybir.AluOpType.mult,
                    op1=mybir.AluOpType.add,
                )
                nc.vector.memset(out_t[:, 1 : 3 * T], 0.0)
                (nc.gpsimd if dma_split else nc.sync).dma_start(
                    out=out_ap[r0 : r0 + SUPER].rearrange(
                        "(p t) c -> p t c", p=BLOCK
                    ),
                    in_=out_t.rearrange("p (t c) -> p t c", c=3),
                )
                return

            # sigmoid(rgb) per channel -> dense per-channel tiles (ScalarE)
            if "sigmoid" in skip:
                g_views = [rgbo4[:, :, :, c] for c in range(3)]
            else:
                g_views = []
                for c in range(3):
                    g_c = p_g.tile([BLOCK, S * T], F32, tag=f"g{c}")
                    nc.scalar.activation(
                        g_c.rearrange("p (t s) -> p t s", t=T),
                        rgbo4[:, :, :, c],
                        mybir.ActivationFunctionType.Sigmoid,
                    )
                    g_views.append(g_c.rearrange("p (t s) -> p t s", t=T))

            if "dm" in skip:
                m_t = depth_t
            else:
                eng_dm = nc.gpsimd if gpsimd_delta_m else nc.vector
                delta_t = p_mid.tile([BLOCK, S * T], F32, tag="delta")
                delta3 = delta_t.rearrange("p (t s) -> p t s", t=T)
                eng_dm.tensor_sub(
                    delta3[:, :, 0 : S - 1], depth3[:, :, 1:S], depth3[:, :, 0 : S - 1]
                )
                m_t = p_mid.tile([BLOCK, S * T], F32, tag="m")
                m3 = m_t.rearrange("p (t s) -> p t s", t=T)
                eng_dm.tensor_mul(
                    m3[:, :, 0 : S - 1],
                    delta3[:, :, 0 : S - 1],
                    rgbo4[:, :, 0 : S - 1, 3],
                )
                eng_dm.tensor_scalar_mul(
                    m3[:, :, S - 1], rgbo4[:, :, S - 1, 3], 1.0e9
                )

            if "scan" in skip:
                cs_t = m_t
            else:
                cs_t = p_mid.tile([BLOCK, S * T], F32, tag="cs")
                for t in range(T):
                    nc.vector.tensor_tensor_scan(
                        cs_t[:, t * S : (t + 1) * S],
                        m_t[:, t * S : (t + 1) * S],
                        m_t[:, t * S : (t + 1) * S],
                        0.0,
                        mybir.AluOpType.add,
                        mybir.AluOpType.bypass,
                    )

            te_t = p_mid.tile([BLOCK, U * T], F32, tag="te")
            te3 = te_t.rearrange("p (t u) -> p t u", t=T)
            nc.vector.memset(te3[:, :, 0:1], 1.0)
            nc.scalar.activation(
                te3[:, :, 1 : S + 1],
                cs_t.rearrange("p (t s) -> p t s", t=T),
                mybir.ActivationFunctionType.Exp,
                scale=-1.0,
            )

            if "w" in skip:
                w_t = te_t
                w_block = lambda t: w_t[:, t * U : t * U + S]
            else:
                w_t = p_mid.tile([BLOCK, S * T], F32, tag="w")
                w3 = w_t.rearrange("p (t s) -> p t s", t=T)
                nc.vector.tensor_sub(w3, te3[:, :, 0:S], te3[:, :, 1 : S + 1])
                w_block = lambda t: w_t[:, t * S : (t + 1) * S]

            out_t = p_out.tile([BLOCK, 3 * T], F32, tag="out")
            if "stt" in skip:
                nc.vector.memset(out_t[:], 0.0)
            else:
                for t in range(T):
                    for c in range(3):
                        acc = out_t[:, t * 3 + c : t * 3 + c + 1]
                        eng = nc.vector if c < 3 - gpsimd_channels else nc.gpsimd
                        tag = "scr" if c < 3 - gpsimd_channels else "scrg"
                        scr = p_scr.tile([BLOCK, S], F32, tag=tag)
                        eng.scalar_tensor_tensor(
                            out=scr[:],
                            in0=w_block(t),
                            scalar=0.0,
                            in1=g_views[c][:, t],
                            op0=mybir.AluOpType.bypass,
                            op1=mybir.AluOpType.mult,
                            accum_out=acc,
                        )
            (nc.gpsimd if dma_split else nc.sync).dma_start(
                out=out_ap[r0 : r0 + SUPER].rearrange("(p t) c -> p t c", p=BLOCK),
                in_=out_t.rearrange("p (t c) -> p t c", c=3),
            )

        def emit_all():
            for sb in range(n_super * repeat):
                emit_superblock((sb % n_super) * SUPER)

        if loop_iters:
            with tc.For_i(0, loop_iters, 1) as _i:
                emit_all()
        else:
            emit_all()
    nc.compile()
    return nc


_NC_CACHE: dict = {}


def _get_nc():
    if "nc" not in _NC_CACHE:
        _NC_CACHE["nc"] = build_nerf_bass()
    return _NC_CACHE["nc"]


def kernel(rgbo: np.ndarray, depth: np.ndarray, **run_kwargs) -> np.ndarray:
    rgbo = np.ascontiguousarray(rgbo, dtype=np.float32)
    depth = np.ascontiguousarray(depth, dtype=np.float32)
    assert rgbo.shape == (N_RAYS, S, 4) and depth.shape == (N_RAYS, S)

    nc = _get_nc()
    in_maps = []
    for i in range(N_CORES):
        sl = slice(i * NC_RAYS, (i + 1) * NC_RAYS)
        in_maps.append({"rgbo": rgbo[sl], "depth": depth[sl]})
    res = run_bass_kernel_spmd(nc, in_maps, core_ids=list(range(N_CORES)), **run_kwargs)
    out = np.concatenate([r["out"] for r in res.results], axis=0)
    if run_kwargs:
        kernel.last_results = res  # stash for profiling harnesses
    return out

